# revision 1
# baseline (speedup 1.0000x reference)
"""Trainium2 Bass kernel for nn_AttentionBlock (B=4, C=1024, T=1024, H=16).

Sharding: data-parallel over batch (4) x sequence-parallel over T (2 halves)
= 8 cores, zero collectives. k/v are computed for the full sequence on every
core (attention needs all keys); q/softmax/attention-output/projection only
for the core's T-half. Per-core inputs are T-permuted on the host so the
SPMD program always works on columns [0, 512).

Numerics: matmuls run in float32r (TF32-like, ~1.5e-4 scale-relative error,
full PE speed) with fp32 PSUM accumulation; everything else is fp32.
"""

import numpy as np

C, T, TH = 1024, 1024, 512
H, CH = 16, 64
NG, GS = 32, 32  # groups, channels per group
EPS = 1e-5
B = 4
NCORES = 8
SCALE2 = 1.0 / 8.0  # (ch^-0.25)^2 applied to q.k scores

_NC = None  # compiled Bass module cache
_LAST_RESULTS = None


def _build_bass():
    import concourse.bacc as bacc
    import concourse.tile as tile
    from concourse import mybir
    from contextlib import ExitStack

    F32 = mybir.dt.float32
    F32R = mybir.dt.float32r
    AF = mybir.ActivationFunctionType
    nc = bacc.Bacc(None, target_bir_lowering=False)

    x_d = nc.dram_tensor("x", [C, T], F32, kind="ExternalInput")
    wq_d = nc.dram_tensor("wq", [C, C], F32R, kind="ExternalInput")
    wk_d = nc.dram_tensor("wk", [C, C], F32R, kind="ExternalInput")
    wv_d = nc.dram_tensor("wv", [C, C], F32R, kind="ExternalInput")
    pw_d = nc.dram_tensor("pw", [C, C], F32R, kind="ExternalInput")
    bq_d = nc.dram_tensor("bq", [C], F32, kind="ExternalInput")
    bk_d = nc.dram_tensor("bk", [C], F32, kind="ExternalInput")
    bv_d = nc.dram_tensor("bv", [C], F32, kind="ExternalInput")
    pb_d = nc.dram_tensor("pb", [C], F32, kind="ExternalInput")
    nw_d = nc.dram_tensor("nw", [C], F32, kind="ExternalInput")
    nb_d = nc.dram_tensor("nb", [C], F32, kind="ExternalInput")
    comb_d = nc.dram_tensor("comb", [128, 4], F32, kind="ExternalInput")
    gbc_d = nc.dram_tensor("gbc", [4, 128], F32, kind="ExternalInput")
    k16_d = nc.dram_tensor("k16", [8, TH], F32R, kind="ExternalInput")
    vones_d = nc.dram_tensor("vones", [128, 8, 1], F32R, kind="ExternalInput")
    prow_d = nc.dram_tensor("prow", [1, TH], F32R, kind="ExternalInput")
    out_d = nc.dram_tensor("out", [C, TH], F32, kind="ExternalOutput")

    with tile.TileContext(nc) as tc, ExitStack() as glob:
        gpool = glob.enter_context(tc.tile_pool(name="gpool", bufs=1))

        # ---- persistent tiles -------------------------------------------
        xnstack = glob.enter_context(ExitStack())
        xnpool = xnstack.enter_context(tc.tile_pool(name="xnpool", bufs=1, side="right"))
        xn = [xnpool.tile([128, T], F32R, name=f"xn{i}", tag=f"xn{i}") for i in range(8)]
        a_all = [gpool.tile([128, TH], F32R, name=f"aall{i}", tag=f"aall{i}") for i in range(8)]
        d16p = [gpool.tile([8, TH], F32, name=f"d16p{p}", tag=f"d16p{p}") for p in range(2)]
        recipp = [gpool.tile([8, TH], F32R, name=f"recipp{p}", tag=f"recipp{p}") for p in range(2)]
        comb_s = gpool.tile([128, 4], F32, name="comb_s")
        gbc_s = gpool.tile([4, 128], F32, name="gbc_s")
        k16_s = gpool.tile([8, TH], F32R, name="k16_s")
        eps4 = gpool.tile([4, 1], F32, name="eps4")
        nc.sync.dma_start(out=comb_s, in_=comb_d[:, :])
        nc.sync.dma_start(out=gbc_s, in_=gbc_d[:, :])
        nc.sync.dma_start(out=k16_s, in_=k16_d[:, :])
        nc.vector.memset(eps4, EPS)

        # ---- phase 1: GroupNorm -> xn (float32r) ------------------------
        # per-tile bn_stats, then ONE batched pipeline for all 8 tiles'
        # group statistics (small DVE ops have large fixed costs)
        with ExitStack() as ph1:
            xpool = ph1.enter_context(tc.tile_pool(name="xpool", bufs=1))
            spool = ph1.enter_context(tc.tile_pool(name="spool", bufs=3))
            gn_ps = ph1.enter_context(tc.tile_pool(name="gn_ps", bufs=1, space="PSUM"))
            bc_ps = ph1.enter_context(tc.tile_pool(name="bc_ps", bufs=1, space="PSUM"))
            mv_all = spool.tile([128, 8, 2], F32, name="mv_all")
            nw_all = spool.tile([128, 8], F32, name="nw_all")
            nb_all = spool.tile([128, 8], F32, name="nb_all")
            nc.sync.dma_start(out=nw_all, in_=nw_d.ap().rearrange("(i p) -> p i", p=128))
            nc.sync.dma_start(out=nb_all, in_=nb_d.ap().rearrange("(i p) -> p i", p=128))
            x_ts = []
            for i in range(8):
                r0 = 128 * i
                x_t = xpool.tile([128, T], F32, tag=f"x_t{i}", name=f"x_t{i}")
                for ch in range(4):
                    nc.sync.dma_start(out=x_t[:, 256 * ch:256 * ch + 256],
                                      in_=x_d[r0:r0 + 128, 256 * ch:256 * ch + 256])
                st = spool.tile([128, 2, 6], F32, tag="st", name=f"st{i}")
                nc.vector.bn_stats(out=st[:, 0, :], in_=x_t[:, 0:512])
                nc.vector.bn_stats(out=st[:, 1, :], in_=x_t[:, 512:1024])
                nc.vector.bn_aggr(out=mv_all[:, i, :], in_=st)
                x_ts.append(x_t)
            # mq = [mean, E[x^2]] per channel, all tiles at once
            mq_all = spool.tile([128, 8, 2], F32, name="mq_all")
            nc.vector.tensor_mul(mq_all[:, :, 1:2], mv_all[:, :, 0:1], mv_all[:, :, 0:1])
            nc.vector.tensor_add(mq_all[:, :, 1:2], mq_all[:, :, 1:2], mv_all[:, :, 1:2])
            nc.vector.tensor_copy(mq_all[:, :, 0:1], mv_all[:, :, 0:1])
            gst = gn_ps.tile([4, 16], F32, name="gst")
            nc.tensor.matmul(gst, comb_s, mq_all.rearrange("p a b -> p (a b)"),
                             start=True, stop=True)
            gsb = spool.tile([4, 8, 2], F32, name="gsb")
            nc.vector.tensor_copy(gsb.rearrange("p a b -> p (a b)"), gst)
            bc_in = spool.tile([4, 8, 2], F32, name="bc_in")
            var4 = spool.tile([4, 8, 1], F32, name="var4")
            nc.vector.tensor_mul(var4, gsb[:, :, 0:1], gsb[:, :, 0:1])
            nc.vector.tensor_sub(var4, gsb[:, :, 1:2], var4)
            nc.scalar.activation(out=bc_in[:, :, 1:2], in_=var4, func=AF.Sqrt,
                                 bias=eps4, scale=1.0)
            nc.vector.reciprocal(bc_in[:, :, 1:2], bc_in[:, :, 1:2])
            nc.vector.tensor_copy(bc_in[:, :, 0:1], gsb[:, :, 0:1])
            bc = bc_ps.tile([128, 8, 2], F32, name="bc")
            nc.tensor.matmul(bc.rearrange("p a b -> p (a b)"), gbc_s,
                             bc_in.rearrange("p a b -> p (a b)"),
                             start=True, stop=True)
            sca = spool.tile([128, 8], F32, name="sca")
            sha = spool.tile([128, 8], F32, name="sha")
            nc.vector.tensor_mul(sca, bc[:, :, 1], nw_all)
            nc.vector.tensor_mul(sha, bc[:, :, 0], sca)
            nc.vector.tensor_sub(sha, nb_all, sha)
            for i in range(8):
                nc.vector.tensor_scalar(out=xn[i], in0=x_ts[i],
                                        scalar1=sca[:, i:i + 1],
                                        scalar2=sha[:, i:i + 1],
                                        op0=mybir.AluOpType.mult,
                                        op1=mybir.AluOpType.add)

        # ---- phase 2: QKV + attention, two passes of 8 heads ------------
        with ExitStack() as ph2:
            wpool = ph2.enter_context(tc.tile_pool(name="wpool", bufs=1))
            qkpool = ph2.enter_context(tc.tile_pool(name="qkpool", bufs=1))
            vpool = ph2.enter_context(tc.tile_pool(name="vpool", bufs=1))
            epool = ph2.enter_context(tc.tile_pool(name="epool", bufs=5))
            bpool = ph2.enter_context(tc.tile_pool(name="bpool", bufs=4))
            dspool = ph2.enter_context(tc.tile_pool(name="dspool", bufs=2))

            # fp32r matmuls cannot use tile_position (row/col packing), so:
            # - k is stored zero-padded per head (K=128 contraction with the
            #   other head's rows zeroed; q stays packed per pair)
            # - v carries a fused ones column per head for the softmax
            #   denominator; the attention-output matmul runs full-width and
            #   the invalid half of its output is simply not copied out
            kpadA = [qkpool.tile([128, T], F32R, tag=f"kpA{j}", name=f"kpA{j}")
                     for j in range(4)]
            kpadB = [qkpool.tile([128, T], F32R, tag=f"kpB{j}", name=f"kpB{j}")
                     for j in range(4)]
            vaA = [vpool.tile([128, 4, 65], F32R, tag=f"vaA{t}", name=f"vaA{t}")
                   for t in range(8)]
            vaB = [vpool.tile([128, 4, 128], F32R, tag=f"vaB{t}", name=f"vaB{t}")
                   for t in range(8)]
            # memset cannot produce fp32r; stage zeros/ones in fp32 scratch
            # tiles and tensor_copy (a valid fp32r rounding producer)
            with tc.tile_pool(name="zpool", bufs=1) as zpool:
                zsrc = zpool.tile([128, T], F32, name="zsrc")
                osrc = zpool.tile([128, 8], F32, name="osrc")
                nc.vector.memset(zsrc, 0.0)
                nc.vector.memset(osrc, 1.0)
                for j in range(4):
                    nc.vector.tensor_copy(kpadA[j][64:128, :], zsrc[64:128, :])
                    nc.vector.tensor_copy(kpadB[j][0:64, :], zsrc[0:64, :])
                for t in range(8):
                    nc.vector.tensor_copy(
                        vaA[t][:, :, 64:65],
                        osrc[:, 0:4].rearrange("p (a b) -> p a b", b=1))
                    nc.vector.tensor_copy(
                        vaB[t][:, :, 0:64],
                        zsrc[:, 0:256].rearrange("p (a b) -> p a b", b=64))
                    nc.vector.tensor_copy(
                        vaB[t][:, :, 0:1],
                        osrc[:, 0:4].rearrange("p (a b) -> p a b", b=1))

            for ps in range(2):
                o0 = ps * TH  # global channel offset of this pass's heads
                with ExitStack() as phq:
                    mm_ps = phq.enter_context(
                        tc.tile_pool(name="mm_ps", bufs=2, space="PSUM"))
                    wq_b = wpool.tile([128, 8, TH], F32R, tag="wq_b", name=f"wq_b{ps}")
                    wk_b = wpool.tile([128, 8, TH], F32R, tag="wk_b", name=f"wk_b{ps}")
                    wv_b = wpool.tile([128, 8, TH], F32R, tag="wv_b", name=f"wv_b{ps}")
                    for c in range(8):
                        nc.sync.dma_start(out=wq_b[:, c, :], in_=wq_d[128 * c:128 * c + 128, o0:o0 + TH])
                        nc.sync.dma_start(out=wk_b[:, c, :], in_=wk_d[128 * c:128 * c + 128, o0:o0 + TH])
                        nc.sync.dma_start(out=wv_b[:, c, :], in_=wv_d[128 * c:128 * c + 128, o0:o0 + TH])
                    if ps == 1:
                        # normalize pass-A attention outputs now; overlaps
                        # with pass-B qkv/attention on PE
                        with nc.allow_low_precision(reason="fp32r feed for PE broadcast"):
                            nc.vector.reciprocal(recipp[0], d16p[0])
                        for atl in range(4):
                            dbc = mm_ps.tile([128, TH], F32, tag="dbc", name=f"dbcA{atl}")
                            nc.tensor.matmul(dbc, k16_s[:, 128 * atl:128 * atl + 128],
                                             recipp[0], start=True, stop=True)
                            nc.vector.tensor_mul(a_all[atl], a_all[atl], dbc)
                            bvt = bpool.tile([128, 1], F32, tag="bvt", name=f"bvtA{atl}")
                            nc.sync.dma_start(out=bvt, in_=bv_d[128 * atl:128 * atl + 128].rearrange("(p one) -> p one", one=1))
                            nc.vector.tensor_scalar_add(a_all[atl], a_all[atl], bvt)

                    q_s = []
                    for ot in range(4):
                        r0 = o0 + 128 * ot
                        qp = mm_ps.tile([128, TH], F32, tag="qp", name=f"qp{ps}_{ot}")
                        for c in range(8):
                            nc.tensor.matmul(qp, wq_b[:, c, 128 * ot:128 * ot + 128],
                                             xn[c][:, 0:TH],
                                             start=(c == 0), stop=(c == 7))
                        bqt = bpool.tile([128, 1], F32, tag="bqt", name=f"bqt{ps}_{ot}")
                        nc.sync.dma_start(out=bqt, in_=bq_d[r0:r0 + 128].rearrange("(p one) -> p one", one=1))
                        qt = qkpool.tile([128, TH], F32R, tag=f"qt{ot}", name=f"qt{ps}_{ot}")
                        nc.vector.tensor_scalar_add(qt, qp, bqt)
                        q_s.append(qt)

                        bkt = bpool.tile([128, 1], F32, tag="bkt", name=f"bkt{ps}_{ot}")
                        nc.sync.dma_start(out=bkt, in_=bk_d[r0:r0 + 128].rearrange("(p one) -> p one", one=1))
                        for sn in range(2):
                            kp = mm_ps.tile([128, TH], F32, tag="kp", name=f"kp{ps}_{ot}_{sn}")
                            for c in range(8):
                                nc.tensor.matmul(kp, wk_b[:, c, 128 * ot:128 * ot + 128],
                                                 xn[c][:, TH * sn:TH * sn + TH],
                                                 start=(c == 0), stop=(c == 7))
                            nc.vector.tensor_scalar_add(
                                kpadA[ot][0:64, TH * sn:TH * sn + TH],
                                kp[0:64, :], bkt[0:64, :])
                            nc.vector.tensor_scalar_add(
                                kpadB[ot][64:128, TH * sn:TH * sn + TH],
                                kp[64:128, :], bkt[64:128, :])

                    for tt in range(8):
                        vp = mm_ps.tile([128, TH], F32, tag="vp", name=f"vp{ps}_{tt}")
                        for c in range(8):
                            nc.tensor.matmul(vp, xn[c][:, 128 * tt:128 * tt + 128],
                                             wv_b[:, c, :],
                                             start=(c == 0), stop=(c == 7))
                        vpv = vp.rearrange("p (h c) -> p h c", c=64)
                        nc.vector.tensor_copy(vaA[tt][:, :, 0:64], vpv[:, 0::2, :])
                        nc.vector.tensor_copy(vaB[tt][:, :, 64:128], vpv[:, 1::2, :])

                if ps == 1:
                    # xn is dead after the last QKV matmul; free its SBUF and
                    # prefetch the projection weights during attention-B
                    xnstack.close()
                    pwpool = glob.enter_context(tc.tile_pool(name="pwpool", bufs=1, side="right"))
                    pw_b = pwpool.tile([128, 8, C], F32R, name="pw_b")
                    for c in range(8):
                        nc.sync.dma_start(out=pw_b[:, c, :],
                                          in_=pw_d[128 * c:128 * c + 128, :])

                # attention for this pass's 4 head-pairs
                with ExitStack() as pha:
                    qk_ps = pha.enter_context(
                        tc.tile_pool(name="qk_ps", bufs=2, space="PSUM"))
                    av_ps = pha.enter_context(
                        tc.tile_pool(name="av_ps", bufs=2, space="PSUM"))
                    for j in range(4):
                        exp_t = []
                        for sc in range(8):
                            qk = qk_ps.tile([128, 2 * TH], F32, tag="qk", name=f"qk{ps}_{j}_{sc}")
                            nc.tensor.matmul(qk[:, 0:TH],
                                             kpadA[j][:, 128 * sc:128 * sc + 128],
                                             q_s[j],
                                             start=True, stop=True)
                            nc.tensor.matmul(qk[:, TH:2 * TH],
                                             kpadB[j][:, 128 * sc:128 * sc + 128],
                                             q_s[j],
                                             start=True, stop=True)
                            et = epool.tile([128, 2 * TH], F32R, tag="et", name=f"et{ps}_{j}_{sc}")
                            nc.scalar.activation(out=et, in_=qk, func=AF.Exp, scale=SCALE2)
                            exp_t.append(et)
                        # attention output: full-width matmuls; head A's
                        # result lands on partitions 0:64 (+ denom at 64),
                        # head B's on 64:128 (+ denom at 0); the rest is
                        # discarded
                        avA = av_ps.tile([128, TH], F32, tag="avA", name=f"avA{ps}_{j}")
                        avB = av_ps.tile([128, TH], F32, tag="avB", name=f"avB{ps}_{j}")
                        for sc in range(8):
                            nc.tensor.matmul(avA[0:65, :], vaA[sc][:, j, :],
                                             exp_t[sc][:, 0:TH],
                                             start=(sc == 0), stop=(sc == 7))
                            nc.tensor.matmul(avB, vaB[sc][:, j, :],
                                             exp_t[sc][:, TH:2 * TH],
                                             start=(sc == 0), stop=(sc == 7))
                        at_ = a_all[4 * ps + j]
                        nc.vector.tensor_copy(at_[0:64, :], avA[0:64, :])
                        nc.vector.tensor_copy(at_[64:128, :], avB[64:128, :])
                        hA, hB = 2 * j, 2 * j + 1  # pass-local head index
                        dsA = dspool.tile([65, TH], F32, tag="dsA", name=f"dsA{ps}_{j}")
                        nc.vector.tensor_copy(dsA[64:65, :], avA[64:65, :])
                        nc.sync.dma_start(out=d16p[ps][hA:hA + 1, :], in_=dsA[64:65, :])
                        dsB = dspool.tile([1, TH], F32, tag="dsB", name=f"dsB{ps}_{j}")
                        nc.vector.tensor_copy(dsB, avB[0:1, :])
                        nc.sync.dma_start(out=d16p[ps][hB:hB + 1, :], in_=dsB)

        # ---- phase 3: normalize, project, residual ----------------------
        with ExitStack() as ph3:
            opool = ph3.enter_context(tc.tile_pool(name="opool", bufs=3))
            xrpool = ph3.enter_context(tc.tile_pool(name="xrpool", bufs=3))
            bc_ps3 = ph3.enter_context(tc.tile_pool(name="bc_ps3", bufs=2, space="PSUM"))
            pj_ps = ph3.enter_context(tc.tile_pool(name="pj_ps", bufs=4, space="PSUM"))

            with nc.allow_low_precision(reason="fp32r feed for PE broadcast"):
                nc.vector.reciprocal(recipp[1], d16p[1])
            for atl in range(4):
                at = 4 + atl
                dbc = bc_ps3.tile([128, TH], F32, tag="dbc", name=f"dbc{at}")
                nc.tensor.matmul(dbc, k16_s[:, 128 * atl:128 * atl + 128],
                                 recipp[1], start=True, stop=True)
                nc.vector.tensor_mul(a_all[at], a_all[at], dbc)
                bvt = xrpool.tile([128, 1], F32, tag="bvt", name=f"bvt{at}")
                nc.sync.dma_start(out=bvt, in_=bv_d[128 * at:128 * at + 128].rearrange("(p one) -> p one", one=1))
                nc.vector.tensor_scalar_add(a_all[at], a_all[at], bvt)

            for ot in range(8):
                r0 = 128 * ot
                hp = pj_ps.tile([128, TH], F32, tag="hp", name=f"hp{ot}")
                for c in range(8):
                    nc.tensor.matmul(hp, pw_b[:, c, r0:r0 + 128],
                                     a_all[c],
                                     start=(c == 0), stop=(c == 7))
                xr = xrpool.tile([128, TH], F32, tag="xr", name=f"xr{ot}")
                nc.sync.dma_start(out=xr, in_=x_d[r0:r0 + 128, 0:TH])
                pbt = xrpool.tile([128, 1], F32, tag="pbt", name=f"pbt{ot}")
                nc.sync.dma_start(out=pbt, in_=pb_d[r0:r0 + 128].rearrange("(p one) -> p one", one=1))
                o_t = opool.tile([128, TH], F32, tag="o_t", name=f"o_t{ot}")
                nc.vector.scalar_tensor_tensor(o_t, hp, pbt, xr,
                                               op0=mybir.AluOpType.add,
                                               op1=mybir.AluOpType.add)
                nc.sync.dma_start(out=out_d[r0:r0 + 128, :], in_=o_t)

    nc.finalize()
    return nc


def kernel(x, norm_weight, norm_bias, qkv_w, qkv_b, proj_w, proj_b):
    from concourse.bass_utils import run_bass_kernel_spmd

    global _NC
    if _NC is None:
        _NC = _build_bass()

    x = np.ascontiguousarray(np.asarray(x, dtype=np.float32))
    nw = np.asarray(norm_weight, np.float32)
    nb = np.asarray(norm_bias, np.float32)
    qw = np.asarray(qkv_w, np.float32).reshape(H, 3, CH, C)
    qb = np.asarray(qkv_b, np.float32).reshape(H, 3, CH)
    pw = np.asarray(proj_w, np.float32)
    pb = np.asarray(proj_b, np.float32)

    wq = np.ascontiguousarray(qw[:, 0].reshape(C, C).T)
    wk = np.ascontiguousarray(qw[:, 1].reshape(C, C).T)
    wv = np.ascontiguousarray(qw[:, 2].reshape(C, C).T)
    bq = np.ascontiguousarray(qb[:, 0].reshape(C))
    bk = np.ascontiguousarray(qb[:, 1].reshape(C))
    bv = np.ascontiguousarray(qb[:, 2].reshape(C))
    pwT = np.ascontiguousarray(pw.T)

    comb = np.zeros((128, 4), np.float32)
    for p in range(128):
        comb[p, p // 32] = 1.0 / 32.0
    gbc = np.zeros((4, 128), np.float32)
    for p in range(128):
        gbc[p // 32, p] = 1.0
    k16 = np.zeros((8, TH), np.float32)
    for atl in range(4):
        for p in range(128):
            k16[2 * atl + (p // 64), 128 * atl + p] = 1.0
    vones = np.ones((128, 8, 1), np.float32)
    prow = np.ones((1, TH), np.float32)

    shared = dict(wq=wq, wk=wk, wv=wv, pw=pwT, bq=bq, bk=bk, bv=bv,
                  pb=pb, nw=nw, nb=nb, comb=comb, gbc=gbc,
                  k16=k16, vones=vones, prow=prow)
    in_maps = []
    for core in range(NCORES):
        b, half = divmod(core, 2)
        xb = x[b] if half == 0 else np.ascontiguousarray(
            np.concatenate([x[b][:, TH:], x[b][:, :TH]], axis=1))
        in_maps.append(dict(x=xb, **shared))

    import os
    kw = {}
    if os.environ.get("BASS_KERNEL_TRACE"):
        cores = os.environ.get("BASS_KERNEL_TRACE_CORES", "0")
        kw = dict(trace=True,
                  trace_cores=[int(c) for c in cores.split(",")],
                  stitch_traces=len(cores.split(",")) > 1)
    res = run_bass_kernel_spmd(_NC, in_maps, core_ids=list(range(NCORES)), **kw)
    global _LAST_RESULTS
    _LAST_RESULTS = res
    out = np.empty((B, C, T), np.float32)
    for core in range(NCORES):
        b, half = divmod(core, 2)
        out[b][:, half * TH:(half + 1) * TH] = res.results[core]["out"]
    return out

